# revision 1
# baseline (speedup 1.0000x reference)
"""Trainium2 Bass kernel for nn_CoupledAttention (sparse_attention).

Strategy (data-parallel over batch N=8, one batch element per core):
  - All attention biases (temporal MLP bias + hop MLP bias) are computed on
    host (tiny MLPs) and folded into the QK^T matmul as extra contraction
    rows (K = 32 qk + 25 hop-onehot + 64 frame-onehot = 121), so the PE
    produces fully-biased logits directly:  S^T[tk, tq] (pre-scaled by
    SCALE/2; the exp applies scale=2).
  - Softmax runs without max-subtraction (logits are provably tiny) in the
    S^T orientation; denominators are obtained by appending a ones-column to
    the value matrix in the PV matmul, so the same matmul emits both
    unnormalized context and row sums.
  - PSUM row layout per head pair (PE out base partition must be 0/32/64 with
    quadrant-limited span; engine operand bases must be 32-aligned):
    h_even lhsT [v|1] M=33 -> rows 0:33 (d0 at 32);
    h_odd  lhsT [1|v] M=33 -> rows 64:97 (d1 at 64, v 65:97);
    a K=1 zero-opener defines the gap rows 32:64 (once per physical bank).
    Two one-row reciprocals (bases 32 and 64), two K=1 ones-row matmuls
    broadcast 1/d over rows 0:97, and a single mul+add writes the
    normalized context + relational term straight into per-pair SBUF
    output tiles (no partition-shifting DMAs anywhere in the steady
    state).
  - The relational block-diagonal term  einsum('hpq,nhfqc->nhfpc', outer, v)
    uses v re-projected directly in 100-token (4-frame) groups (16 extra
    small matmuls instead of 84 SBUF->SBUF regroup DMAs), against a constant
    block-diagonal [100,100] matrix.
  - Final: Y^T = sum_ip projt_ip.T @ outt_ip (3 accumulating K=97 matmuls
    per output panel, emitted per-panel as soon as all three pair combines
    land; junk rows 32:65 of outt are killed by zero weight rows on the
    host side).

Layout notes: tokens padded 1600 -> 1664 = 13*128 chunks; padded x columns
are zero so padded keys contribute exp(0)=1 * v=0 and ones-col=0 (no effect).
tq is processed in panels of <=500 (PSUM bank, multiple of 100).
"""

import numpy as np

H = 6
P = 25
F = 64
T = 1600
TPAD = 1664
DIM = 192
HD = 32
NB = 8
SCALE = HD ** -0.5
S2 = SCALE * 0.5
NCHUNK = TPAD // 128  # 13
KAUG = 89   # 25 hop onehot + 64 frame onehot
KFULL = 121
PAIRS = [(0, 1), (2, 3), (4, 5)]
# (col_start, width) tq panels: each <= 512 (one PSUM bank per matmul) and a
# multiple of 100 (rel chunk granularity). Padded cols 1600:1664 are never
# touched by attention (only by the q/k/v projections).
PANELS = [(0, 500), (500, 500), (1000, 500), (1500, 100)]

_CACHE = {}

# fused-exp polynomial (p(w)^2 ~ exp(2w), |w| <= 0.42): minimax-fitted
C3V, C2V = 0.16654227731456347, 0.50569975039442
# chunks handled by the DVE-path fused exp (rest go to ScalarE exp)
DVE_CHUNKS = frozenset({7, 8, 9, 10, 11, 12})


def _expb_ref(in0, in1, s0, s1, imm2):
    a = np.asarray(in0, np.float32)
    if in1 is not None:
        a = a + np.asarray(in1, np.float32).reshape(a.shape)
    p = ((s0 * a + s1) * a + 1.0) * a + 1.0
    return p * p


def _register_expb():
    """Register the fused exp DveOp (idempotent; runtime OPS append)."""
    from concourse.dve_spec import Spec, Src0, Src1, C0, C1, One, lower, sq
    from concourse.dve_uop import DveOpSpec
    import concourse.dve_ops as dmod
    from concourse.dve_ops import DveOp, OPS
    for op in OPS:
        if op.name == "EXPB_ANT":
            return op
    w = Src0 + Src1
    spec = Spec(body=sq(((C0 * w + C1) * w + One) * w + One), reference=_expb_ref)
    shas = {}
    for ver in ("v3", "v4"):
        try:
            s = DveOpSpec(name="EXPB_ANT", opcode=1, uops=lower(spec, ver=ver),
                          rd1_en=True)
            shas[ver] = s.sha(ver)
        except Exception:
            pass
    op = DveOp("EXPB_ANT", spec, subdim=False, uops_sha=shas)
    OPS.append(op)
    dmod._SUB_OPCODE_FOR_NAME[op.name] = dmod._CUSTOM_DVE_ROW_BASE + len(OPS) - 1
    dmod.CUSTOM_DVE_SPECS[op.name] = spec
    return op


def _f32(x):
    return np.ascontiguousarray(x, dtype=np.float32)


def _bf16(x):
    import ml_dtypes
    return np.ascontiguousarray(np.asarray(x, dtype=np.float32).astype(ml_dtypes.bfloat16))


def _host_prep(inputs):
    """Compute bias tables and all device input arrays on host (numpy f32)."""
    x = _f32(inputs["x"])              # (8, 1600, 192)
    qkv_w = _f32(inputs["qkv_w"])      # (576, 192)
    proj_w = _f32(inputs["proj_w"])    # (192, 192)
    proj_b = _f32(inputs["proj_b"])    # (192,)
    t_w1 = _f32(inputs["t_w1"]); t_b1 = _f32(inputs["t_b1"])
    t_w2 = _f32(inputs["t_w2"]); t_b2 = _f32(inputs["t_b2"])
    h_w1 = _f32(inputs["h_w1"]); h_b1 = _f32(inputs["h_b1"])
    h_w2 = _f32(inputs["h_w2"]); h_b2 = _f32(inputs["h_b2"])
    outer = _f32(inputs["outer"])      # (H, P, P)
    alpha = float(np.asarray(inputs["alpha"]).reshape(-1)[0])
    hop = np.asarray(inputs["hop"])    # (P, P) int32

    # --- bias tables (exactly as in reference, f32) ---
    rel = (np.arange(2 * F - 1, dtype=np.float32) - (F - 1))[:, None]   # (127,1)
    tab = np.maximum(rel @ t_w1.T + t_b1, 0.0) @ t_w2.T + t_b2          # (127, H)
    hf = hop.astype(np.float32).reshape(-1, 1)
    hb = (np.maximum(hf @ h_w1.T + h_b1, 0.0) @ h_w2.T + h_b2).reshape(P, P, H)

    tq = np.arange(TPAD)
    fidx = tq // P          # frame index (0..63 valid; >=64 for pads)
    pidx = tq % P

    # --- augmented K rows ---
    # kaug[a, tk] = [p(tk)==a] (25 rows), then [f(tk)==m] (64 rows); pads zero
    kaug = np.zeros((KAUG, TPAD), np.float32)
    for a in range(P):
        kaug[a, :T] = (pidx[:T] == a)
    for m in range(F):
        kaug[P + m, :T] = (fidx[:T] == m)

    # qaug[h, a, tq] = S2*hb[p(tq), a, h];  qaug[h, 25+m, tq] = S2*tab[f(tq)-m+63, h]
    qaug = np.zeros((H, KAUG, TPAD), np.float32)
    for h in range(H):
        qaug[h, :P, :T] = S2 * hb[pidx[:T], :, h].T                 # (25, 1600)
        dmat = fidx[:T][None, :] - np.arange(F)[:, None] + (F - 1)  # (64, 1600)
        qaug[h, P:, :T] = S2 * tab[dmat, h]

    # tabpat[h, p, c, i] = S2*tab[i - f(128c+p) + 63, h] — the Src1 pattern
    # tiles for the DVE-path fused exp (temporal bias via repeat-AP); stored
    # in device layout [128, NCHUNK, F] so the load is contiguous.
    tabpat = np.zeros((H, 128, NCHUNK, F), np.float32)
    fgrid = fidx.reshape(NCHUNK, 128)                    # frame of each (c, p)
    for h in range(H):
        for c in range(NCHUNK):
            idx = np.arange(F)[None, :] - fgrid[c][:, None] + (F - 1)  # (128, 64)
            valid = fgrid[c][:, None] < F
            tabpat[h, :, c, :] = np.where(
                valid, S2 * tab[np.clip(idx, 0, 2 * F - 2), h], 0.0)

    # --- projection weights (transposed, padded) ---
    # wq/wk: lhsT [e, c] chunks; M padded to 256 (cols 192:256 zero)
    def wchunks(w, scale):
        wt = np.zeros((DIM, 256), np.float32)
        wt[:, :DIM] = scale * w.T          # [e, c]
        a = np.zeros((128, 256), np.float32); a[:, :] = wt[:128]
        b = np.zeros((128, 256), np.float32); b[:64, :] = wt[128:]
        return a, b

    wq_a, wq_b = wchunks(qkv_w[0:DIM], S2)
    wk_a, wk_b = wchunks(qkv_w[DIM:2 * DIM], 1.0)
    # wv: rhs [e, c] (N=192), v scaled by alpha
    vscale = alpha if alpha != 0.0 else 1.0
    wvt = vscale * qkv_w[2 * DIM:3 * DIM].T     # (192, 192) [e, c]
    wv_a = np.zeros((128, DIM), np.float32); wv_a[:, :] = wvt[:128]
    wv_b = np.zeros((128, DIM), np.float32); wv_b[:64, :] = wvt[128:]

    # --- rel block-diagonal matrix (shared across heads; compensate alpha) ---
    out0 = outer[0]
    oscale = (1.0 / alpha) if alpha != 0.0 else 1.0
    oblk = np.zeros((100, 100), np.float32)
    for b in range(4):
        # rhs[j=tk_local, n=tq_local] = outer[p(tq), p(tk)]
        oblk[b * P:(b + 1) * P, b * P:(b + 1) * P] = oscale * out0.T
    head_indep = all(np.allclose(outer[0], outer[h]) for h in range(H))

    # --- out projection: 3 lhsT tiles [97, 256], one per head pair ---
    # row r of outt_ip: r in 0:32 -> (h=2ip, dim r); r in 32:65 -> junk
    # (zero weight rows); r in 65:97 -> (h=2ip+1, dim r-65)
    pt = proj_w.T                                # [input c, output d]
    projts = []
    for ip in range(3):
        m = np.zeros((97, 256), np.float32)
        h0, h1 = 2 * ip, 2 * ip + 1
        m[0:32, :DIM] = pt[32 * h0:32 * h0 + 32, :]
        m[65:97, :DIM] = pt[32 * h1:32 * h1 + 32, :]
        projts.append(m)
    pb_a = np.zeros((128, 1), np.float32); pb_a[:, 0] = proj_b[:128]
    pb_b = np.zeros((128, 1), np.float32); pb_b[:64, 0] = proj_b[128:]



    common = {
        "kaug": _bf16(kaug),
        "qaug": _bf16(qaug),
        "tabpat": _bf16(tabpat),
        "wq_a": _bf16(wq_a), "wq_b": _bf16(wq_b),
        "wk_a": _bf16(wk_a), "wk_b": _bf16(wk_b),
        "wv_a": _bf16(wv_a), "wv_b": _bf16(wv_b),
        "oblk": _bf16(oblk),
        "projt_0": _bf16(projts[0]), "projt_1": _bf16(projts[1]),
        "projt_2": _bf16(projts[2]),
        "pb_a": _f32(pb_a), "pb_b": _f32(pb_b),
    }
    # per-core x^T padded
    xts = []
    for n in range(NB):
        xt = np.zeros((DIM, TPAD), np.float32)
        xt[:, :T] = x[n].T
        xa = np.zeros((128, TPAD), np.float32); xa[:, :] = xt[:128]
        xb = np.zeros((128, TPAD), np.float32); xb[:64, :] = xt[128:]
        xts.append((_bf16(xa), _bf16(xb)))
    return common, xts, alpha, head_indep


def _build_program(alpha, loop_n=None):
    """Emit the Bass/Tile program (data independent; alpha affects a branch)."""
    from contextlib import ExitStack
    import concourse.bass as bass
    import concourse.bacc as bacc
    import concourse.tile as tile
    from concourse import mybir

    BF = mybir.dt.bfloat16
    FP = mybir.dt.float32
    EXP = mybir.ActivationFunctionType.Exp
    IDENT = mybir.ActivationFunctionType.Identity

    EXPB = _register_expb()
    nc = bacc.Bacc("TRN2", target_bir_lowering=False, debug=False,
                   enable_asserts=False)

    def din(name, shape, dt=BF):
        return nc.dram_tensor(name, list(shape), dt, kind="ExternalInput").ap()

    d_xa = din("xt_a", (128, TPAD)); d_xb = din("xt_b", (128, TPAD))
    d_kaug = din("kaug", (KAUG, TPAD))
    d_qaug = din("qaug", (H, KAUG, TPAD))
    d_wqa = din("wq_a", (128, 256)); d_wqb = din("wq_b", (128, 256))
    d_wka = din("wk_a", (128, 256)); d_wkb = din("wk_b", (128, 256))
    d_wva = din("wv_a", (128, DIM)); d_wvb = din("wv_b", (128, DIM))
    d_oblk = din("oblk", (100, 100))
    d_tabpat = din("tabpat", (H, 128, NCHUNK, F))
    d_projt = [din(f"projt_{i}", (97, 256)) for i in range(3)]
    d_pba = din("pb_a", (128, 1), FP); d_pbb = din("pb_b", (128, 1), FP)
    d_ya = nc.dram_tensor("y_a", [128, T], FP, kind="ExternalOutput").ap()
    d_yb = nc.dram_tensor("y_b", [64, T], FP, kind="ExternalOutput").ap()

    # projection panels cover the padded token range, one PSUM bank each
    PPANELS = [(0, 512), (512, 512), (1024, 512), (1536, 128)]

    with tile.TileContext(nc) as tc, ExitStack() as ctx:
        singles = ctx.enter_context(tc.tile_pool(name="singles", bufs=1))
        psum = ctx.enter_context(tc.tile_pool(name="psum", bufs=1, space="PSUM"))
        ppool = ctx.enter_context(tc.tile_pool(name="ppool", bufs=28))
        cpool = ctx.enter_context(tc.tile_pool(name="cpool", bufs=2))

        if loop_n is not None:
            # benchmarking mode: run the whole body loop_n times on-device
            ctx.enter_context(tc.For_i(0, loop_n, 1))

        # ---- load constants ----
        # big x tiles + tab patterns on the gpsimd SWDGE queue; small weights
        # on the sync HWDGE queue (parallel prologue)
        xa = singles.tile([128, TPAD], BF); nc.gpsimd.dma_start(out=xa, in_=d_xa)
        xb = singles.tile([128, TPAD], BF); nc.gpsimd.dma_start(out=xb, in_=d_xb)
        wqa = singles.tile([128, 256], BF); nc.sync.dma_start(out=wqa, in_=d_wqa)
        wqb = singles.tile([128, 256], BF); nc.sync.dma_start(out=wqb, in_=d_wqb)
        wka = singles.tile([128, 256], BF); nc.sync.dma_start(out=wka, in_=d_wka)
        wkb = singles.tile([128, 256], BF); nc.sync.dma_start(out=wkb, in_=d_wkb)
        wva = singles.tile([128, DIM], BF); nc.sync.dma_start(out=wva, in_=d_wva)
        wvb = singles.tile([128, DIM], BF); nc.sync.dma_start(out=wvb, in_=d_wvb)
        oblkt = singles.tile([100, 100], BF); nc.sync.dma_start(out=oblkt, in_=d_oblk)
        projt = []
        for i in range(3):
            pti = singles.tile([97, 256], BF, name=f"projt{i}")
            nc.sync.dma_start(out=pti, in_=d_projt[i])
            projt.append(pti)
        pba = singles.tile([128, 1], FP); nc.sync.dma_start(out=pba, in_=d_pba)
        pbb = singles.tile([128, 1], FP); nc.sync.dma_start(out=pbb, in_=d_pbb)

        # ---- staging + per-head tiles ----
        qt4 = singles.tile([128, TPAD], BF)   # q^T heads 0..3 (rows 32h..)
        qt2 = singles.tile([64, TPAD], BF)    # heads 4,5
        kt4 = singles.tile([128, TPAD], BF)
        kt2 = singles.tile([64, TPAD], BF)
        # v token-major, pair-packed: per (chunk, pair) 66 cols =
        # [v_h0 (0:32) | 1 (32) | 1 (33) | v_h1 (34:66)]
        # h_even PV lhsT = cols 0:33 ([v|1]); h_odd = cols 33:66 ([1|v])
        vall = singles.tile([128, NCHUNK, 3, 66], BF)
        kfull = [singles.tile([128, TPAD], BF, name=f"kfull{h}") for h in range(H)]
        qfull = [singles.tile([128, TPAD], BF, name=f"qfull{h}") for h in range(H)]
        # v in 100-token (4-frame) groups for the rel term: [100p, 16g, 192]
        vrelp = singles.tile([128, 16, DIM], BF)
        outt = [singles.tile([97, T], BF, name=f"outt{i}") for i in range(3)]
        # zero operands for the PSUM gap-row openers (rows 32:64) and the
        # ones row for the 1/d broadcast matmuls
        zrow = singles.tile([1, 512], BF)
        nc.vector.memset(zrow, 0.0)
        zcol = singles.tile([1, 32], BF)
        nc.vector.memset(zcol, 0.0)
        o64 = singles.tile([1, 64], BF)
        nc.vector.memset(o64, 1.0)
        o33 = singles.tile([33, 64], BF)   # ones row at partition 32 (must
        nc.vector.memset(o33[32:33, :], 1.0)  # match the rcb[32] rhs base)

        # ---- q^T / k^T projections: out[c, t] = w.T @ x^T ----
        for (wa, wb, st4, st2) in ((wqa, wqb, qt4, qt2), (wka, wkb, kt4, kt2)):
            for mset in range(2):
                for (c0, w) in PPANELS:
                    ps = psum.tile([128, 512], FP, tag="s", bufs=4, name="ps_proj")
                    nc.tensor.matmul(ps[:, 0:w],
                                     wa[:, mset * 128:mset * 128 + 128],
                                     xa[:, c0:c0 + w], start=True, stop=False)
                    nc.tensor.matmul(ps[:, 0:w],
                                     wb[:, mset * 128:mset * 128 + 128],
                                     xb[:, c0:c0 + w], start=False, stop=True)
                    if mset == 0:
                        nc.scalar.copy(st4[:, c0:c0 + w], ps[:, 0:w])
                    else:
                        nc.scalar.copy(st2[:, c0:c0 + w], ps[0:64, 0:w])

        # ---- v projection (token-major, alpha-scaled) ----
        for c in range(NCHUNK):
            ps = psum.tile([128, DIM], FP, tag="s", bufs=4, name="ps_v")
            nc.tensor.matmul(ps, xa[:, c * 128:(c + 1) * 128], wva,
                             start=True, stop=False)
            nc.tensor.matmul(ps, xb[:, c * 128:(c + 1) * 128], wvb,
                             start=False, stop=True)
            # scatter heads into pair-packed layout: (ip, hi, dim)
            dst = bass.AP(
                tensor=vall.tensor,
                offset=vall.offset + c * (3 * 66),
                ap=[list(vall.ap[0]), [66, 3], [34, 2], [1, 32]])
            nc.scalar.copy(dst, ps.rearrange("p (a b c) -> p a b c", b=2, c=32))
        # ones columns (zero for padded tokens)
        for c in range(NCHUNK):
            rows = 64 if c == NCHUNK - 1 else 128
            nc.vector.memset(vall[0:rows, c, :, 32:34], 1.0)
            if rows < 128:
                nc.vector.memset(vall[rows:128, c, :, 32:34], 0.0)

        # ---- v re-projection in 100-token groups (for the rel term) ----
        for g in range(16):
            t0 = 100 * g
            ps = psum.tile([128, DIM], FP, tag="s", bufs=4, name="ps_vg")
            nc.tensor.matmul(ps[0:100, :], xa[:, t0:t0 + 100], wva,
                             start=True, stop=False)
            nc.tensor.matmul(ps[0:100, :], xb[:, t0:t0 + 100], wvb,
                             start=False, stop=True)
            nc.scalar.copy(vrelp[0:100, g, :], ps[0:100, :])

        # ---- assemble per-head augmented q/k tiles ----
        # even heads: rows [k(0:32) | hopOH(32:57) | tempOH(57:121)]
        # odd heads:  rows [tempOH(0:64) | k(64:96) | hopOH(96:121)]
        # (odd layout keeps k+hop contiguous at base 64 for the row-tiled
        #  DVE-path QK; the K=121 contraction order is irrelevant as long as
        #  k-side and q-side rows pair up)
        for h in range(H):
            ksrc = kt4[32 * h:32 * h + 32, :] if h < 4 else \
                   kt2[32 * (h - 4):32 * (h - 4) + 32, :]
            qsrc = qt4[32 * h:32 * h + 32, :] if h < 4 else \
                   qt2[32 * (h - 4):32 * (h - 4) + 32, :]
            if h % 2 == 0:
                nc.gpsimd.dma_start(out=kfull[h][0:32, :], in_=ksrc)
                nc.gpsimd.dma_start(out=qfull[h][0:32, :], in_=qsrc)
                nc.scalar.dma_start(out=kfull[h][32:32 + KAUG, :], in_=d_kaug)
                nc.scalar.dma_start(out=qfull[h][32:32 + KAUG, :], in_=d_qaug[h])
            else:
                nc.sync.dma_start(out=kfull[h][0:64, :], in_=d_kaug[P:KAUG, :])
                nc.sync.dma_start(out=qfull[h][0:64, :], in_=d_qaug[h, P:KAUG, :])
                nc.gpsimd.dma_start(out=kfull[h][64:96, :], in_=ksrc)
                nc.gpsimd.dma_start(out=qfull[h][64:96, :], in_=qsrc)
                nc.sync.dma_start(out=kfull[h][96:121, :], in_=d_kaug[0:P, :])
                nc.sync.dma_start(out=qfull[h][96:121, :], in_=d_qaug[h, 0:P, :])
        tabsb = [singles.tile([128, NCHUNK, F], BF, name=f"tabsb{h}") for h in range(H)]
        for h in range(H):
            nc.gpsimd.dma_start(out=tabsb[h], in_=d_tabpat[h])

        # ---- attention blocks (skewed pipeline, panel-major so each
        # output-projection panel can start as soon as its three pair
        # combines are done) ----
        blocks = [(ip, ih) for ih in range(len(PANELS)) for ip in range(3)]
        state = {}

        def tab_ap(h, c, c0, w):
            # Src1 repeat-pattern: element [p, j] reads tabsb[h][p, c, (c0+j)//25]
            base = tabsb[h]
            return bass.AP(
                tensor=base.tensor,
                offset=base.offset + c * F + c0 // P,
                ap=[[base.ap[0][0], 128], [1, w // P], [0, P]])

        def emit_qk_exp(blk):
            ip, ih = blk
            h0, h1 = PAIRS[ip]
            c0, w = PANELS[ih]
            ptiles = {}
            # phase A: ScalarE-exp chunks (bias fully in-matmul, K=121)
            for c in range(NCHUNK):
                if c in DVE_CHUNKS:
                    continue
                for h in (h0, h1):
                    ps = psum.tile([128, 512], FP, tag="s", bufs=4, name="ps_s")
                    nc.tensor.matmul(ps[:, 0:w],
                                     kfull[h][0:KFULL, c * 128:(c + 1) * 128],
                                     qfull[h][0:KFULL, c0:c0 + w],
                                     start=True, stop=True)
                    pt = ppool.tile([128, 512], BF, tag="p", name="ptile")
                    nc.scalar.activation(pt[:, 0:w], ps[:, 0:w], EXP, scale=2.0)
                    ptiles[(c, h)] = pt
            # phase B: DVE-path chunks — 2-head row-tiled QK (K=57, hop
            # in-matmul) + fused poly-exp with the temporal bias via Src1
            for c in range(NCHUNK):
                if c not in DVE_CHUNKS:
                    continue
                psA = psum.tile([128, 512], FP, tag="s", bufs=4, name="ps_s")
                psB = psum.tile([128, 512], FP, tag="s", bufs=4, name="ps_s")
                nc.tensor.matmul(psA[:, 0:w],
                                 kfull[h0][0:57, c * 128:(c + 1) * 128],
                                 qfull[h0][0:57, c0:c0 + w],
                                 start=True, stop=True)
                nc.tensor.matmul(psB[:, 0:w],
                                 kfull[h1][64:121, c * 128:(c + 1) * 128],
                                 qfull[h1][64:121, c0:c0 + w],
                                 start=True, stop=True)
                for h, ps in ((h0, psA), (h1, psB)):
                    pt = ppool.tile([128, 512], BF, tag="p", name="ptile")
                    nc.vector._custom_dve(
                        EXPB,
                        out=pt[:, 0:w].rearrange("p (a b) -> p a b", b=P),
                        in0=ps[:, 0:w].rearrange("p (a b) -> p a b", b=P),
                        in1=tab_ap(h, c, c0, w),
                        s0=C3V, s1=C2V)
                    ptiles[(c, h)] = pt
            state[blk] = {"ptiles": ptiles}

        opened = [0]

        def emit_pv_rel(blk):
            ip, ih = blk
            h0, h1 = PAIRS[ip]
            c0, w = PANELS[ih]
            ptiles = state[blk]["ptiles"]
            ctx_ps = psum.tile([128, 512], FP, tag="ctx", bufs=2, name="ps_ctx")
            # gap opener (zeros rows 32:64; h_even start then re-zeros its
            # own 0:33 incl row 32), h_even -> rows 0:33, h_odd -> 64:97.
            # Rows 33:64 are never disturbed afterwards, so each physical
            # bank only needs the opener once (full width).
            if opened[0] < 2:
                nc.tensor.matmul(ctx_ps[32:64, :], zcol, zrow,
                                 start=True, stop=True, skip_group_check=True)
            for c in range(NCHUNK):
                for hi in range(2):
                    pt = ptiles[(c, (h0, h1)[hi])]
                    nc.tensor.matmul(
                        ctx_ps[64 * hi:64 * hi + 33, 0:w],
                        vall[:, c, ip, 33 * hi:33 * hi + 33],
                        pt[:, 0:w],
                        start=(c == 0), stop=(c == NCHUNK - 1),
                        skip_group_check=True)
            rel_ps = psum.tile([128, 512], FP, tag="rel", bufs=1, name="ps_rel")
            if opened[0] < 1:
                nc.tensor.matmul(rel_ps[32:64, :], zcol, zrow,
                                 start=True, stop=True, skip_group_check=True)
            opened[0] += 1
            for g in range(w // 100):
                gg = c0 // 100 + g
                for hi, h in ((0, h0), (1, h1)):
                    # even: M=33 cols [v_h | junk] -> rows 0:33
                    # odd:  M=33 cols [junk | v_h] -> rows 64:97
                    # (junk lands in the dead d-rows 32 / 64)
                    if hi == 0:
                        nc.tensor.matmul(
                            rel_ps[0:33, g * 100:(g + 1) * 100],
                            vrelp[0:100, gg, 32 * h:32 * h + 33],
                            oblkt, start=True, stop=True,
                            skip_group_check=True)
                    else:
                        nc.tensor.matmul(
                            rel_ps[64:97, g * 100:(g + 1) * 100],
                            vrelp[0:100, gg, 32 * h - 1:32 * h + 32],
                            oblkt, start=True, stop=True,
                            skip_group_check=True)
            state[blk]["ctx"] = ctx_ps
            state[blk]["rel"] = rel_ps

        def emit_combine(blk):
            ip, ih = blk
            c0, w = PANELS[ih]
            ctx_ps = state[blk]["ctx"]; rel_ps = state[blk]["rel"]
            if alpha != 0.0:
                # 1/d per head (engine bases must be 32-aligned: rcb rows
                # 0 and 32), then two K=1 ones-row matmuls broadcast 1/d
                # over rows 0:64 / 64:97
                rcb = cpool.tile([33, 512], BF, tag="rcb", name="rcb")
                with nc.allow_low_precision(
                        reason="bf16 1/d: 2^-8 rel err, well within tolerance"):
                    nc.vector.reciprocal(rcb[0:1, 0:w], ctx_ps[32:33, 0:w])
                    nc.vector.reciprocal(rcb[32:33, 0:w], ctx_ps[64:65, 0:w])
                rb_ps = psum.tile([128, 512], FP, tag="rb", bufs=1, name="ps_rb")
                nc.tensor.matmul(rb_ps[0:64, 0:w], o64, rcb[0:1, 0:w],
                                 start=True, stop=True, skip_group_check=True)
                nc.tensor.matmul(rb_ps[64:97, 0:w], o33[32:33, 0:33],
                                 rcb[32:33, 0:w],
                                 start=True, stop=True, skip_group_check=True)
                # TensorTensor can't read two PSUM operands; stage rb in SBUF
                rb_sb = cpool.tile([97, 512], BF, tag="rbsb", name="rbsb")
                nc.scalar.copy(rb_sb[:, 0:w], rb_ps[0:97, 0:w])
                t1 = cpool.tile([97, 512], BF, tag="t1", name="t1")
                nc.vector.tensor_mul(t1[:, 0:w], ctx_ps[0:97, 0:w],
                                     rb_sb[:, 0:w])
                nc.vector.tensor_add(outt[ip][0:97, c0:c0 + w],
                                     rel_ps[0:97, 0:w], t1[:, 0:w])
            else:
                nc.scalar.copy(outt[ip][0:97, c0:c0 + w], rel_ps[0:97, 0:w])

        def emit_proj_panel(ih):
            # Y^T[d, c0:c0+w] = sum_ip projt_ip.T @ outt_ip[:, c0:c0+w]
            c0, w = PANELS[ih]
            for mset in range(2):
                ps = psum.tile([128, 512], FP, tag="s", bufs=4, name="ps_y")
                for i in range(3):
                    nc.tensor.matmul(ps[:, 0:w],
                                     projt[i][:, mset * 128:mset * 128 + 128],
                                     outt[i][:, c0:c0 + w],
                                     start=(i == 0), stop=(i == 2))
                ysb = cpool.tile([128, 512], FP, tag="ysb", name="ysb")
                pb = pba if mset == 0 else pbb
                nc.scalar.activation(ysb[:, 0:w], ps[:, 0:w], IDENT,
                                     bias=pb, scale=1.0)
                if mset == 0:
                    nc.sync.dma_start(out=d_ya[:, c0:c0 + w], in_=ysb[:, 0:w])
                else:
                    nc.sync.dma_start(out=d_yb[:, c0:c0 + w], in_=ysb[0:64, 0:w])

        for i, blk in enumerate(blocks):
            emit_qk_exp(blk)
            if i > 0:
                prev = blocks[i - 1]
                emit_pv_rel(prev)
                emit_combine(prev)
                if prev[0] == 2:
                    emit_proj_panel(prev[1])
        emit_pv_rel(blocks[-1])
        emit_combine(blocks[-1])
        emit_proj_panel(blocks[-1][1])

    nc.compile()
    return nc


def kernel(**inputs):
    common, xts, alpha, head_indep = _host_prep(inputs)
    assert head_indep, "outer must be head-independent (np.tile in reference)"

    key = ("prog", alpha == 0.0)
    if key not in _CACHE:
        _CACHE[key] = _build_program(alpha)
    nc = _CACHE[key]

    in_maps = []
    for n in range(NB):
        m = dict(common)
        m["xt_a"], m["xt_b"] = xts[n]
        in_maps.append(m)

    from concourse.bass_utils import run_bass_kernel_spmd
    res = run_bass_kernel_spmd(nc, in_maps, core_ids=list(range(NB)))
    out = np.zeros((NB, T, DIM), np.float32)
    for n in range(NB):
        ya = np.asarray(res.results[n]["y_a"], np.float32)   # (128, 1600)
        yb = np.asarray(res.results[n]["y_b"], np.float32)   # (64, 1600)
        out[n] = np.concatenate([ya, yb], axis=0).T
    return out

